# revision 51
# baseline (speedup 1.0000x reference)
"""Trainium2 Bass kernel for nn_Attention_30305289240928.

Single-layer causal attention with RMSNorm prologue:
    xn = x * rsqrt(mean(x^2) + eps)           (RMSNorm, no weight)
    qkv = xn @ wqkv.T  -> per-head q, k, v    (16 heads, head_dim 128)
    out = softmax(causal(q k^T / sqrt(128))) v, concat heads, @ wo.T

Sharding: head-parallel tensor parallel over 8 NeuronCores.
Core c owns heads 2c, 2c+1 (wqkv rows c*768:(c+1)*768) and the matching
wo input-columns c*256:(c+1)*256. Each core computes a full-shape partial
of the output projection (rank-256 contribution); the host sums the 8
partials (the TP all-reduce, done host-side at gather time).

Device-side design:
  - All matmuls in float32r (TF32-like, full PE rate at N>=256);
    measured end-to-end relative error ~3e-4.
  - The RMSNorm scale s[t] factors out of the projection: QKV is computed
    from RAW x, then s is folded into Q (free-dim broadcast multiply at
    PSUM eviction), into the exp() per-partition scale (s[kt]/sqrt(D)),
    and into V (per-partition multiply at eviction).
  - Scores are computed transposed, S.T[kt, qt], so the softmax-exp output
    feeds the PV matmul directly (kt on partitions) with no transposes.
    Causal masking = per-block N-sliced matmuls + one 128x128 triangular
    multiplicative mask on diagonal blocks; below-diagonal blocks are
    never computed.
  - sum-of-exp via ones-matmul accumulated in PSUM alongside PV;
    1/sumexp via single-pass Newton reciprocal on DVE.
  - DMA instruction count is managed against HWDGE descriptor-gen time
    (~0.6us/instruction): per-chunk DMAs only for the latency-critical
    tb=0 ramp, half-block batches for later xt loads, and grouped 2-row-
    block output writes. Output projection is interleaved one query-block
    behind attention so the softmax-normalize chain and the 16.8MB output
    DMA stay off the TensorE critical path.
"""

import numpy as np

import concourse.bacc as bacc
import concourse.mybir as mybir
import concourse.tile as tile
from concourse import bass_utils

# Problem shapes (hardcoded per contract)
S = 2048          # sequence length
H = 2048          # hidden
NH = 16           # heads
D = 128           # head dim
EPS = 1e-5
N_CORES = 8
HPC = NH // N_CORES        # heads per core = 2
FPC = 3 * D * HPC          # wqkv features per core = 768
CPC = D * HPC              # attn dims (wo input cols) per core = 256

TB = 256                   # token block width (phase 1)
NTB = S // TB              # 8
NM = TB // 128             # 128-wide sub-blocks per token block
NHO = H // 128             # 16 hidden 128-chunks
QB = 512                   # query block width (phase 2)
NQB = S // QB              # 4
NKB = S // 128             # 16 key 128-blocks
SQRT_D_INV = 1.0 / float(np.sqrt(D))

f32 = mybir.dt.float32
f32r = mybir.dt.float32r

_CACHED_NC = None


def _build():
    nc = bacc.Bacc("TRN2", target_bir_lowering=False, debug=False,
                   num_devices=N_CORES)
    xT_d = nc.dram_tensor("xT", [H, S], f32, kind="ExternalInput").ap()
    wT_d = nc.dram_tensor("wT", [H, FPC], f32, kind="ExternalInput").ap()
    woT_d = nc.dram_tensor("woT", [CPC, S], f32, kind="ExternalInput").ap()
    # cst = [ones(128,128) | zeros(128,128) | tri_upper(128,128) | eye(128,128)]
    cst_d = nc.dram_tensor("cst", [128, 512], f32, kind="ExternalInput").ap()
    outT_d = nc.dram_tensor("outT", [H, S], f32, kind="ExternalOutput").ap()

    with tile.TileContext(nc) as tc:
        with tc.tile_pool(name="const", bufs=1) as const_pool, \
             tc.tile_pool(name="qk", bufs=1) as qk_pool, \
             tc.tile_pool(name="vsb", bufs=1) as v_pool, \
             tc.tile_pool(name="attn", bufs=1) as attn_pool, \
             tc.tile_pool(name="svec", bufs=1) as s_pool:

            ones_r = const_pool.tile([128, 128], f32r, tag="ones")
            zt = const_pool.tile([128, 256], f32, tag="zt")   # [zeros | tri]
            tri = zt[:, 128:256]
            eye = const_pool.tile([128, 128], f32, tag="eye")
            eps_b = const_pool.tile([128, 1], f32, tag="eps")
            nc.gpsimd.memset(eps_b[:], EPS)

            # phase-1 outputs (live into phases 2/3)
            qkT = qk_pool.tile([128, 2 * HPC, S], f32r)   # [q0,k0,q1,k1] x S
            v_sb = v_pool.tile([128, NKB, CPC], f32r)     # V natural, t-chunked
            attnT = attn_pool.tile([128, HPC, S], f32r)   # O.T rows (this core)
            s_bc = s_pool.tile([128, NTB, TB], f32)       # s[t] bcast over parts
            sTd = s_pool.tile([128, NKB], f32)            # s[t]/sqrt(D), t on parts
            sT = s_pool.tile([128, NKB], f32)             # s[t] plain, t on parts

            # ---------------- Phase 1: RMSNorm stats + QKV projection ------
            with tc.tile_pool(name="wt", bufs=1) as wt_pool, \
                 tc.tile_pool(name="xt", bufs=2) as xt_pool, \
                 tc.tile_pool(name="sq", bufs=3) as sq_pool, \
                 tc.tile_pool(name="ph1", bufs=2) as ph1_pool, \
                 tc.tile_pool(name="ps_qk", bufs=4, space="PSUM") as psum_qk, \
                 tc.tile_pool(name="ps_v", bufs=2, space="PSUM") as psum_v, \
                 tc.tile_pool(name="ps_ssq", bufs=1, space="PSUM") as psum_ssq, \
                 tc.tile_pool(name="ps_t", bufs=1, space="PSUM") as psum_t:

                def load_xt(tb):
                    # two half-batched DMAs per token block: few HWDGE
                    # descriptor-gen slots, but the first half still lands
                    # early enough to start the ho-serial chains
                    chunks = []
                    for half in range(2):
                        t = xt_pool.tile([128, NHO // 2, TB], f32r,
                                         tag=f"xtb{half}")
                        nc.sync.dma_start(
                            t[:],
                            xT_d[half * 1024:(half + 1) * 1024,
                                 tb * TB:(tb + 1) * TB]
                            .rearrange("(ho p) t -> p ho t", p=128)
                            .bitcast(f32r))
                        chunks.extend(t[:, ho] for ho in range(NHO // 2))
                    return chunks

                # interleave xt(tb=0) and wt chunk loads so the first
                # K-matmul chain is DMA-paced with minimal lead time
                xt_cur = []
                wt = []
                for ho in range(NHO):
                    tx = wt_pool.tile([128, TB], f32r, tag=f"xt0_{ho}")
                    nc.sync.dma_start(
                        tx[:], xT_d[ho * 128:(ho + 1) * 128, 0:TB].bitcast(f32r))
                    xt_cur.append(tx)
                    tw = wt_pool.tile([128, FPC], f32r, tag=f"wt{ho}")
                    nc.sync.dma_start(
                        tw[:], wT_d[ho * 128:(ho + 1) * 128, :].bitcast(f32r))
                    wt.append(tw)
                    if ho == 1:
                        # consts slot in behind the first compute chunks
                        nc.sync.dma_start(ones_r[:], cst_d[:, 0:128].bitcast(f32r))
                        nc.sync.dma_start(zt[:], cst_d[:, 128:384])
                        nc.sync.dma_start(eye[:], cst_d[:, 384:512])
                for tb in range(NTB):
                    xt = xt_cur
                    if tb + 1 < NTB:
                        xt_next = load_xt(tb + 1)

                    # squares first: ACT/DVE fill while PE runs K matmuls
                    sqs = []
                    for ho in range(NHO):
                        sq = sq_pool.tile([128, TB], f32r, tag=f"sq{ho % 4}")
                        if ho % 2 == 0:
                            nc.scalar.activation(
                                sq[:], xt[ho][:],
                                mybir.ActivationFunctionType.Square)
                        else:
                            nc.vector.tensor_tensor(
                                sq[:], xt[ho][:].bitcast(f32),
                                xt[ho][:].bitcast(f32), mybir.AluOpType.mult)
                        sqs.append(sq)

                    def qk_block(slot, fb):
                        # qkT slots: 0=q0 1=k0 2=q1 3=k1 ; feature layout per
                        # head: [q(128) k(128) v(128)] x 2 heads
                        ps = psum_qk.tile([128, TB], f32)
                        for ho in range(NHO):
                            nc.tensor.matmul(
                                ps[:], wt[ho][:, fb * 128:(fb + 1) * 128],
                                xt[ho][:], start=(ho == 0), stop=(ho == NHO - 1))
                        dst = qkT[:, slot, tb * TB:(tb + 1) * TB]
                        if slot in (0, 2):   # Q: scale by s[t] during eviction
                            nc.vector.tensor_tensor(dst, ps[:], s_bc[:, tb],
                                                    mybir.AluOpType.mult)
                        else:                # K: plain copy
                            nc.scalar.copy(dst, ps[:])

                    # K blocks (eviction independent of s)
                    qk_block(1, 1)
                    qk_block(3, 4)

                    # sum of squares over hidden (sq tiles all ready by now)
                    ps_ssq = psum_ssq.tile([128, TB], f32)
                    for ho in range(NHO):
                        nc.tensor.matmul(ps_ssq[:], ones_r[:], sqs[ho][:],
                                         start=(ho == 0), stop=(ho == NHO - 1))
                    # s = 1/sqrt(ssq/H + eps)
                    sqrt_t = ph1_pool.tile([128, TB], f32, tag="sqrt")
                    nc.scalar.activation(sqrt_t[:], ps_ssq[:],
                                         mybir.ActivationFunctionType.Sqrt,
                                         bias=eps_b[:], scale=1.0 / H)
                    nc.vector.reciprocal_approx_fast(s_bc[:, tb], sqrt_t[:])

                    # Q blocks (eviction waits on s_bc, ready by now)
                    qk_block(0, 0)
                    qk_block(2, 3)

                    # transpose s into partition-major sT/sTd columns (late:
                    # keeps the ssq->sqrt->recip latency off PE's back)
                    for m in range(NM):
                        pt = psum_t.tile([128, 128], f32)
                        nc.tensor.transpose(pt[:], s_bc[:, tb, m * 128:(m + 1) * 128],
                                            eye[:])
                        col = tb * NM + m
                        nc.scalar.mul(sTd[:, col:col + 1], pt[:, 0:1], SQRT_D_INV)
                        nc.scalar.copy(sT[:, col:col + 1], pt[:, 0:1])

                    # V blocks: out (t, dv) via lhsT = xT chunk, rhs = wv cols
                    for m in range(NM):
                        ps = psum_v.tile([128, CPC], f32)
                        for ho in range(NHO):
                            wv = wt[ho][:].rearrange(
                                "p (hd c f) -> p hd c f", hd=HPC, c=3)[:, :, 2, :]
                            nc.tensor.matmul(
                                ps[:], xt[ho][:, m * 128:(m + 1) * 128],
                                wv, start=(ho == 0), stop=(ho == NHO - 1))
                        chunk = tb * NM + m
                        nc.vector.tensor_scalar_mul(
                            v_sb[:, chunk], ps[:], sT[:, chunk:chunk + 1])

                    if tb + 1 < NTB:
                        xt_cur = xt_next

            # -------- Phase 2+3: attention (qb-outer) + output projection ---
            with tc.tile_pool(name="wo", bufs=1) as wo_pool, \
                 tc.tile_pool(name="exps", bufs=8) as exp_pool, \
                 tc.tile_pool(name="rse", bufs=2) as rse_pool, \
                 tc.tile_pool(name="ostage", bufs=6) as out_pool, \
                 tc.tile_pool(name="ps_s", bufs=3, space="PSUM") as psum_s, \
                 tc.tile_pool(name="ps_o", bufs=2, space="PSUM") as psum_o, \
                 tc.tile_pool(name="ps_se", bufs=1, space="PSUM") as psum_se, \
                 tc.tile_pool(name="ps_out", bufs=2, space="PSUM") as psum_out:
                # wo.T streams in while early attention runs (first use is
                # the qb=0 output-projection block, ~10us into phase 2)
                woT = wo_pool.tile([128, HPC, S], f32r)   # wo.T slice
                nc.sync.dma_start(
                    woT[:], woT_d.rearrange("(ch p) o -> p ch o", p=128)
                    .bitcast(f32r))
                def attn_head(qb, h):
                    kb_hi = (qb + 1) * (QB // 128) - 1
                    if True:
                        q_slot, k_slot = 2 * h, 2 * h + 1
                        po = psum_o.tile([128, QB], f32)
                        pse = psum_se.tile([128, QB], f32)
                        for kb in range(kb_hi + 1):
                            j = kb - qb * (QB // 128)  # >=0 in diagonal zone
                            # j==3 pads the active range to N=256 (fp32r is
                            # 4x slower below 256); the extra below-diagonal
                            # strip is zeroed by the widened [zeros|tri] mask
                            lo = 256 if j == 3 else max(0, j) * 128
                            ps = psum_s.tile([128, QB], f32)
                            nc.tensor.matmul(
                                ps[:, lo:],
                                qkT[:, k_slot, kb * 128:(kb + 1) * 128],
                                qkT[:, q_slot, qb * QB + lo:(qb + 1) * QB],
                                start=True, stop=True)
                            es = exp_pool.tile([128, QB], f32r)
                            nc.scalar.activation(
                                es[:, lo:], ps[:, lo:],
                                mybir.ActivationFunctionType.Exp,
                                scale=sTd[:, kb:kb + 1])
                            if j == 3:
                                nc.vector.tensor_tensor(
                                    es[:, 256:512],
                                    es[:, 256:512].bitcast(f32),
                                    zt[:], mybir.AluOpType.mult)
                            elif j >= 0:
                                nc.vector.tensor_tensor(
                                    es[:, j * 128:(j + 1) * 128],
                                    es[:, j * 128:(j + 1) * 128].bitcast(f32),
                                    tri[:], mybir.AluOpType.mult)
                            nc.tensor.matmul(
                                po[:, lo:], v_sb[:, kb, h * D:(h + 1) * D],
                                es[:, lo:], start=(kb == 0), stop=(kb == kb_hi))
                            nc.tensor.matmul(
                                pse[:, lo:], ones_r[:], es[:, lo:],
                                start=(kb == 0), stop=(kb == kb_hi))
                        rse = rse_pool.tile([128, QB], f32)
                        nc.vector.reciprocal_approx_fast(rse[:], pse[:])
                        nc.vector.tensor_tensor(
                            attnT[:, h, qb * QB:(qb + 1) * QB], po[:], rse[:],
                            mybir.AluOpType.mult)

                def outproj(sb, gs=0, ge=8, borrow=False):
                    # evacs land in a 2-block staging tile; one DMA per group
                    for g in range(gs, ge):
                        st = out_pool.tile([128, 2, 512], f32, tag="ost")
                        for oi in range(2):
                            ob = g * 2 + oi
                            # the score pool is idle during the final block;
                            # borrow its banks to deepen the psum rotation
                            if borrow and ob % 2 == 0:
                                ps = psum_s.tile([128, QB], f32)
                            else:
                                ps = psum_out.tile([128, 512], f32)
                            for ch in range(HPC):
                                nc.tensor.matmul(
                                    ps[:], woT[:, ch, ob * 128:(ob + 1) * 128],
                                    attnT[:, ch, sb * 512:(sb + 1) * 512],
                                    start=(ch == 0), stop=(ch == HPC - 1))
                            if ob % 2 == 0:
                                nc.scalar.copy(st[:, oi], ps[:])
                            else:
                                nc.vector.tensor_copy(st[:, oi], ps[:])
                        nc.sync.dma_start(
                            outT_d[g * 256:(g + 1) * 256,
                                   sb * 512:(sb + 1) * 512]
                            .rearrange("(ob p) t -> p ob t", p=128), st[:])

                # interleave: outproj(qb) emitted after attn(qb+1) h=0 so the
                # pse->recip->attnT chain never sits on PE's critical path
                attn_head(0, 0)
                attn_head(0, 1)
                attn_head(1, 0)
                outproj(0)
                attn_head(1, 1)
                attn_head(2, 0)
                outproj(1)
                attn_head(2, 1)
                attn_head(3, 0)
                outproj(2, 0, 4)
                attn_head(3, 1)
                outproj(2, 4, 8, borrow=True)
                outproj(3, borrow=True)
    nc.compile()
    return nc


def get_nc():
    global _CACHED_NC
    if _CACHED_NC is None:
        _CACHED_NC = _build()
    return _CACHED_NC


def make_in_maps(x, wqkv, wo):
    x = np.asarray(x, dtype=np.float32)
    wqkv = np.asarray(wqkv, dtype=np.float32)
    wo = np.asarray(wo, dtype=np.float32)
    xT = np.ascontiguousarray(x.T)
    cst = np.concatenate(
        [np.ones((128, 128), np.float32),
         np.zeros((128, 128), np.float32),
         np.triu(np.ones((128, 128), np.float32)),
         np.eye(128, dtype=np.float32)], axis=1)
    in_maps = []
    for c in range(N_CORES):
        wT = np.ascontiguousarray(wqkv[c * FPC:(c + 1) * FPC].T)
        woT = np.ascontiguousarray(wo[:, c * CPC:(c + 1) * CPC].T)
        in_maps.append({"xT": xT, "wT": wT, "woT": woT, "cst": cst})
    return in_maps


def kernel(x, wqkv, wo):
    nc = get_nc()
    in_maps = make_in_maps(x, wqkv, wo)
    res = None
    for attempt in range(4):
        try:
            res = bass_utils.run_bass_kernel_spmd(
                nc, in_maps, core_ids=list(range(N_CORES)))
            break
        except Exception:
            # transient NRT device wedges have been observed; they recover
            # after a short quiescent period, so back off before retrying
            if attempt == 3:
                raise
            import time
            time.sleep(20 * (attempt + 1))
    outT = np.zeros((H, S), dtype=np.float32)
    for c in range(N_CORES):
        outT += res.results[c]["outT"]
    return np.ascontiguousarray(outT.T)


# revision 56
# speedup vs baseline: 1.0071x; 1.0071x over previous
"""Trainium2 Bass kernel for nn_Attention_30305289240928.

Single-layer causal attention with RMSNorm prologue:
    xn = x * rsqrt(mean(x^2) + eps)           (RMSNorm, no weight)
    qkv = xn @ wqkv.T  -> per-head q, k, v    (16 heads, head_dim 128)
    out = softmax(causal(q k^T / sqrt(128))) v, concat heads, @ wo.T

Sharding: head-parallel tensor parallel over 8 NeuronCores.
Core c owns heads 2c, 2c+1 (wqkv rows c*768:(c+1)*768) and the matching
wo input-columns c*256:(c+1)*256. Each core computes a full-shape partial
of the output projection (rank-256 contribution); the host sums the 8
partials (the TP all-reduce, done host-side at gather time).

Device-side design:
  - All matmuls in float32r (TF32-like, full PE rate at N>=256);
    measured end-to-end relative error ~3e-4.
  - The RMSNorm scale s[t] factors out of the projection: QKV is computed
    from RAW x, then s is folded into Q (free-dim broadcast multiply at
    PSUM eviction), into the exp() per-partition scale (s[kt]/sqrt(D)),
    and into V (per-partition multiply at eviction).
  - Scores are computed transposed, S.T[kt, qt], so the softmax-exp output
    feeds the PV matmul directly (kt on partitions) with no transposes.
    Causal masking = per-block N-sliced matmuls + one 128x128 triangular
    multiplicative mask on diagonal blocks; below-diagonal blocks are
    never computed.
  - sum-of-exp via ones-matmul accumulated in PSUM alongside PV;
    1/sumexp via single-pass Newton reciprocal on DVE.
  - DMA instruction count is managed against HWDGE descriptor-gen time
    (~0.6us/instruction): per-chunk DMAs only for the latency-critical
    tb=0 ramp, half-block batches for later xt loads, and grouped 2-row-
    block output writes. Output projection is interleaved one query-block
    behind attention so the softmax-normalize chain and the 16.8MB output
    DMA stay off the TensorE critical path.
"""

import numpy as np

import concourse.bacc as bacc
import concourse.mybir as mybir
import concourse.tile as tile
from concourse import bass_utils

# Problem shapes (hardcoded per contract)
S = 2048          # sequence length
H = 2048          # hidden
NH = 16           # heads
D = 128           # head dim
EPS = 1e-5
N_CORES = 8
HPC = NH // N_CORES        # heads per core = 2
FPC = 3 * D * HPC          # wqkv features per core = 768
CPC = D * HPC              # attn dims (wo input cols) per core = 256

TB = 256                   # token block width (phase 1)
NTB = S // TB              # 8
NM = TB // 128             # 128-wide sub-blocks per token block
NHO = H // 128             # 16 hidden 128-chunks
QB = 512                   # query block width (phase 2)
NQB = S // QB              # 4
NKB = S // 128             # 16 key 128-blocks
SQRT_D_INV = 1.0 / float(np.sqrt(D))

f32 = mybir.dt.float32
f32r = mybir.dt.float32r

_CACHED_NC = None


def _build():
    nc = bacc.Bacc("TRN2", target_bir_lowering=False, debug=False,
                   num_devices=N_CORES)
    xT_d = nc.dram_tensor("xT", [H, S], f32, kind="ExternalInput").ap()
    wT_d = nc.dram_tensor("wT", [H, FPC], f32, kind="ExternalInput").ap()
    woT_d = nc.dram_tensor("woT", [CPC, S], f32, kind="ExternalInput").ap()
    # cst = [ones(128,128) | zeros(128,128) | tri_upper(128,128) | eye(128,128)]
    cst_d = nc.dram_tensor("cst", [128, 512], f32, kind="ExternalInput").ap()
    outT_d = nc.dram_tensor("outT", [H, S], f32, kind="ExternalOutput").ap()

    with tile.TileContext(nc) as tc:
        with tc.tile_pool(name="const", bufs=1) as const_pool, \
             tc.tile_pool(name="qk", bufs=1) as qk_pool, \
             tc.tile_pool(name="vsb", bufs=1) as v_pool, \
             tc.tile_pool(name="attn", bufs=1) as attn_pool, \
             tc.tile_pool(name="svec", bufs=1) as s_pool:

            ones_r = const_pool.tile([128, 128], f32r, tag="ones")
            zt = const_pool.tile([128, 256], f32, tag="zt")   # [zeros | tri]
            tri = zt[:, 128:256]
            eye = const_pool.tile([128, 128], f32, tag="eye")
            eps_b = const_pool.tile([128, 1], f32, tag="eps")
            nc.gpsimd.memset(eps_b[:], EPS)

            # phase-1 outputs (live into phases 2/3)
            qkT = qk_pool.tile([128, 2 * HPC, S], f32r)   # [q0,k0,q1,k1] x S
            v_sb = v_pool.tile([128, NKB, CPC], f32r)     # V natural, t-chunked
            attnT = attn_pool.tile([128, HPC, S], f32r)   # O.T rows (this core)
            s_bc = s_pool.tile([128, NTB, TB], f32)       # s[t] bcast over parts
            sTd = s_pool.tile([128, NKB], f32)            # s[t]/sqrt(D), t on parts
            sT = s_pool.tile([128, NKB], f32)             # s[t] plain, t on parts

            # ---------------- Phase 1: RMSNorm stats + QKV projection ------
            with tc.tile_pool(name="wt", bufs=1) as wt_pool, \
                 tc.tile_pool(name="xt", bufs=2) as xt_pool, \
                 tc.tile_pool(name="sq", bufs=3) as sq_pool, \
                 tc.tile_pool(name="ph1", bufs=2) as ph1_pool, \
                 tc.tile_pool(name="ps_qk", bufs=4, space="PSUM") as psum_qk, \
                 tc.tile_pool(name="ps_v", bufs=2, space="PSUM") as psum_v, \
                 tc.tile_pool(name="ps_ssq", bufs=1, space="PSUM") as psum_ssq, \
                 tc.tile_pool(name="ps_t", bufs=1, space="PSUM") as psum_t:

                def load_xt(tb):
                    # two half-batched DMAs per token block: few HWDGE
                    # descriptor-gen slots, but the first half still lands
                    # early enough to start the ho-serial chains
                    chunks = []
                    for half in range(2):
                        t = xt_pool.tile([128, NHO // 2, TB], f32r,
                                         tag=f"xtb{half}")
                        nc.sync.dma_start(
                            t[:],
                            xT_d[half * 1024:(half + 1) * 1024,
                                 tb * TB:(tb + 1) * TB]
                            .rearrange("(ho p) t -> p ho t", p=128)
                            .bitcast(f32r))
                        chunks.extend(t[:, ho] for ho in range(NHO // 2))
                    return chunks

                # interleave xt(tb=0) and wt chunk loads so the first
                # K-matmul chain is DMA-paced with minimal lead time
                xt_cur = []
                wt = []
                for ho in range(NHO):
                    tx = wt_pool.tile([128, TB], f32r, tag=f"xt0_{ho}")
                    nc.sync.dma_start(
                        tx[:], xT_d[ho * 128:(ho + 1) * 128, 0:TB].bitcast(f32r))
                    xt_cur.append(tx)
                    tw = wt_pool.tile([128, FPC], f32r, tag=f"wt{ho}")
                    nc.sync.dma_start(
                        tw[:], wT_d[ho * 128:(ho + 1) * 128, :].bitcast(f32r))
                    wt.append(tw)
                    if ho == 1:
                        # consts slot in behind the first compute chunks
                        nc.sync.dma_start(ones_r[:], cst_d[:, 0:128].bitcast(f32r))
                        nc.sync.dma_start(zt[:], cst_d[:, 128:384])
                        nc.sync.dma_start(eye[:], cst_d[:, 384:512])
                for tb in range(NTB):
                    xt = xt_cur
                    if tb + 1 < NTB:
                        xt_next = load_xt(tb + 1)

                    # squares first: ACT/DVE fill while PE runs K matmuls
                    sqs = []
                    for ho in range(NHO):
                        sq = sq_pool.tile([128, TB], f32r, tag=f"sq{ho % 4}")
                        if ho % 2 == 0:
                            nc.scalar.activation(
                                sq[:], xt[ho][:],
                                mybir.ActivationFunctionType.Square)
                        else:
                            nc.vector.tensor_tensor(
                                sq[:], xt[ho][:].bitcast(f32),
                                xt[ho][:].bitcast(f32), mybir.AluOpType.mult)
                        sqs.append(sq)

                    def qk_block(slot, fb):
                        # qkT slots: 0=q0 1=k0 2=q1 3=k1 ; feature layout per
                        # head: [q(128) k(128) v(128)] x 2 heads
                        ps = psum_qk.tile([128, TB], f32)
                        for ho in range(NHO):
                            nc.tensor.matmul(
                                ps[:], wt[ho][:, fb * 128:(fb + 1) * 128],
                                xt[ho][:], start=(ho == 0), stop=(ho == NHO - 1))
                        dst = qkT[:, slot, tb * TB:(tb + 1) * TB]
                        if slot in (0, 2):   # Q: scale by s[t] during eviction
                            nc.vector.tensor_tensor(dst, ps[:], s_bc[:, tb],
                                                    mybir.AluOpType.mult)
                        else:                # K: plain copy
                            nc.scalar.copy(dst, ps[:])

                    # K blocks (eviction independent of s)
                    qk_block(1, 1)
                    qk_block(3, 4)

                    # sum of squares over hidden (sq tiles all ready by now)
                    ps_ssq = psum_ssq.tile([128, TB], f32)
                    for ho in range(NHO):
                        nc.tensor.matmul(ps_ssq[:], ones_r[:], sqs[ho][:],
                                         start=(ho == 0), stop=(ho == NHO - 1))
                    # s = 1/sqrt(ssq/H + eps)
                    sqrt_t = ph1_pool.tile([128, TB], f32, tag="sqrt")
                    nc.scalar.activation(sqrt_t[:], ps_ssq[:],
                                         mybir.ActivationFunctionType.Sqrt,
                                         bias=eps_b[:], scale=1.0 / H)
                    nc.vector.reciprocal_approx_fast(s_bc[:, tb], sqrt_t[:])

                    # Q blocks (eviction waits on s_bc, ready by now)
                    qk_block(0, 0)
                    qk_block(2, 3)

                    # transpose s into partition-major sT/sTd columns (late:
                    # keeps the ssq->sqrt->recip latency off PE's back)
                    for m in range(NM):
                        pt = psum_t.tile([128, 128], f32)
                        nc.tensor.transpose(pt[:], s_bc[:, tb, m * 128:(m + 1) * 128],
                                            eye[:])
                        col = tb * NM + m
                        nc.scalar.mul(sTd[:, col:col + 1], pt[:, 0:1], SQRT_D_INV)
                        nc.scalar.copy(sT[:, col:col + 1], pt[:, 0:1])

                    # V blocks: out (t, dv) via lhsT = xT chunk, rhs = wv cols
                    for m in range(NM):
                        ps = psum_v.tile([128, CPC], f32)
                        for ho in range(NHO):
                            wv = wt[ho][:].rearrange(
                                "p (hd c f) -> p hd c f", hd=HPC, c=3)[:, :, 2, :]
                            nc.tensor.matmul(
                                ps[:], xt[ho][:, m * 128:(m + 1) * 128],
                                wv, start=(ho == 0), stop=(ho == NHO - 1))
                        chunk = tb * NM + m
                        nc.vector.tensor_scalar_mul(
                            v_sb[:, chunk], ps[:], sT[:, chunk:chunk + 1])

                    if tb + 1 < NTB:
                        xt_cur = xt_next

            # -------- Phase 2+3: attention (qb-outer) + output projection ---
            with tc.tile_pool(name="wo", bufs=1) as wo_pool, \
                 tc.tile_pool(name="exps", bufs=8) as exp_pool, \
                 tc.tile_pool(name="rse", bufs=2) as rse_pool, \
                 tc.tile_pool(name="ostage", bufs=6) as out_pool, \
                 tc.tile_pool(name="ps_s", bufs=3, space="PSUM") as psum_s, \
                 tc.tile_pool(name="ps_o", bufs=2, space="PSUM") as psum_o, \
                 tc.tile_pool(name="ps_se", bufs=1, space="PSUM") as psum_se, \
                 tc.tile_pool(name="ps_out", bufs=2, space="PSUM") as psum_out:
                # wo.T streams in while early attention runs (first use is
                # the qb=0 output-projection block, ~10us into phase 2)
                woT = wo_pool.tile([128, HPC, S], f32r)   # wo.T slice
                nc.sync.dma_start(
                    woT[:], woT_d.rearrange("(ch p) o -> p ch o", p=128)
                    .bitcast(f32r))
                def attn_head(qb, h):
                    kb_hi = (qb + 1) * (QB // 128) - 1
                    if True:
                        q_slot, k_slot = 2 * h, 2 * h + 1
                        po = psum_o.tile([128, QB], f32)
                        pse = psum_se.tile([128, QB], f32)
                        for kb in range(kb_hi + 1):
                            j = kb - qb * (QB // 128)  # >=0 in diagonal zone
                            # j==3 pads the active range to N=256 (fp32r is
                            # 4x slower below 256); the extra below-diagonal
                            # strip is zeroed by the widened [zeros|tri] mask
                            lo = 256 if j == 3 else max(0, j) * 128
                            ps = psum_s.tile([128, QB], f32)
                            nc.tensor.matmul(
                                ps[:, lo:],
                                qkT[:, k_slot, kb * 128:(kb + 1) * 128],
                                qkT[:, q_slot, qb * QB + lo:(qb + 1) * QB],
                                start=True, stop=True)
                            es = exp_pool.tile([128, QB], f32r)
                            nc.scalar.activation(
                                es[:, lo:], ps[:, lo:],
                                mybir.ActivationFunctionType.Exp,
                                scale=sTd[:, kb:kb + 1])
                            if j == 3:
                                nc.vector.tensor_tensor(
                                    es[:, 256:512],
                                    es[:, 256:512].bitcast(f32),
                                    zt[:], mybir.AluOpType.mult)
                            elif j >= 0:
                                nc.vector.tensor_tensor(
                                    es[:, j * 128:(j + 1) * 128],
                                    es[:, j * 128:(j + 1) * 128].bitcast(f32),
                                    tri[:], mybir.AluOpType.mult)
                            nc.tensor.matmul(
                                po[:, lo:], v_sb[:, kb, h * D:(h + 1) * D],
                                es[:, lo:], start=(kb == 0), stop=(kb == kb_hi))
                            nc.tensor.matmul(
                                pse[:, lo:], ones_r[:], es[:, lo:],
                                start=(kb == 0), stop=(kb == kb_hi))
                        rse = rse_pool.tile([128, QB], f32)
                        nc.vector.reciprocal_approx_fast(rse[:], pse[:])
                        nc.vector.tensor_tensor(
                            attnT[:, h, qb * QB:(qb + 1) * QB], po[:], rse[:],
                            mybir.AluOpType.mult)

                def outproj(sb, gs=0, ge=8, borrow=False):
                    # evacs land in a 2-block staging tile; one DMA per group
                    for g in range(gs, ge):
                        st = out_pool.tile([128, 2, 512], f32, tag="ost")
                        for oi in range(2):
                            ob = g * 2 + oi
                            # the score pool is idle during the final block;
                            # borrow its banks to deepen the psum rotation
                            if borrow and ob % 2 == 0:
                                ps = psum_s.tile([128, QB], f32)
                            else:
                                ps = psum_out.tile([128, 512], f32)
                            for ch in range(HPC):
                                nc.tensor.matmul(
                                    ps[:], woT[:, ch, ob * 128:(ob + 1) * 128],
                                    attnT[:, ch, sb * 512:(sb + 1) * 512],
                                    start=(ch == 0), stop=(ch == HPC - 1))
                            if ob % 2 == 0:
                                nc.scalar.copy(st[:, oi], ps[:])
                            else:
                                nc.vector.tensor_copy(st[:, oi], ps[:])
                        nc.sync.dma_start(
                            outT_d[g * 256:(g + 1) * 256,
                                   sb * 512:(sb + 1) * 512]
                            .rearrange("(ob p) t -> p ob t", p=128), st[:])

                # interleave: outproj(qb) emitted after attn(qb+1) h=0 so the
                # pse->recip->attnT chain never sits on PE's critical path
                attn_head(0, 0)
                attn_head(0, 1)
                attn_head(1, 0)
                outproj(0)
                attn_head(1, 1)
                attn_head(2, 0)
                outproj(1)
                attn_head(2, 1)
                attn_head(3, 0)
                outproj(2, 0, 6)
                attn_head(3, 1)
                outproj(2, 6, 8, borrow=True)
                outproj(3, borrow=True)
    nc.compile()
    return nc


def get_nc():
    global _CACHED_NC
    if _CACHED_NC is None:
        _CACHED_NC = _build()
    return _CACHED_NC


def make_in_maps(x, wqkv, wo):
    x = np.asarray(x, dtype=np.float32)
    wqkv = np.asarray(wqkv, dtype=np.float32)
    wo = np.asarray(wo, dtype=np.float32)
    xT = np.ascontiguousarray(x.T)
    cst = np.concatenate(
        [np.ones((128, 128), np.float32),
         np.zeros((128, 128), np.float32),
         np.triu(np.ones((128, 128), np.float32)),
         np.eye(128, dtype=np.float32)], axis=1)
    in_maps = []
    for c in range(N_CORES):
        wT = np.ascontiguousarray(wqkv[c * FPC:(c + 1) * FPC].T)
        woT = np.ascontiguousarray(wo[:, c * CPC:(c + 1) * CPC].T)
        in_maps.append({"xT": xT, "wT": wT, "woT": woT, "cst": cst})
    return in_maps


def kernel(x, wqkv, wo):
    nc = get_nc()
    in_maps = make_in_maps(x, wqkv, wo)
    res = None
    for attempt in range(4):
        try:
            res = bass_utils.run_bass_kernel_spmd(
                nc, in_maps, core_ids=list(range(N_CORES)))
            break
        except Exception:
            # transient NRT device wedges have been observed; they recover
            # after a short quiescent period, so back off before retrying
            if attempt == 3:
                raise
            import time
            time.sleep(20 * (attempt + 1))
    outT = np.zeros((H, S), dtype=np.float32)
    for c in range(N_CORES):
        outT += res.results[c]["outT"]
    return np.ascontiguousarray(outT.T)
